# revision 15
# baseline (speedup 1.0000x reference)
"""CRC24A encoder (nn_CRCEncoder) as a Bass/Tile kernel on 8 Trainium2 NeuronCores.

Computation (per the reference):
    out = concat([X, (X @ G) mod 2], axis=-1)
with X [16384, 4096] of {0,1} float32 and G [4096, 24] of {0,1} float32.

Strategy: pure data parallel over the batch dim — each of the 8 cores gets a
[2048, 4096] shard and the full (replicated) G. Two layout decisions make the
device side a pure streaming kernel at the HBM roofline:

  - 8-bit I/O. Every value is exactly 0.0 or 1.0, which fp8 e4m3 represents
    exactly (0x00 / 0x38), so the device moves 16.8 MiB per core per pass
    instead of 67.3 MiB at f32. The host converts f32 <-> byte codes; DRAM
    tensors are declared uint8 so jax/PJRT never sees an fp8 dtype, and
    device-side APs bitcast to float8e4 where the engines need it.
  - k-major (transposed) layout. The host uploads X.T [4096, 2048] and reads
    back the output k-major [4120, 2048]. Loads then put the contraction dim
    on SBUF partitions directly: no TensorE transposes, no PSUM evacuation
    copies — the parity matmul streams straight from the DMA staging tile,
    and the X passthrough stores bit-verbatim from the same tile. The parity
    (X @ G).T [24, rows] is itself k-major, landing as output rows 4096:4119
    with no transpose-back. Host-side de-transposition happens once per
    kernel() call.

Per 512-row block: 16 DoubleRow fp8 matmuls (contraction 256/instruction,
0.5 cycles/row, moving free dim 512 — the full-win regime) accumulate
(X @ G).T in fp32 PSUM, exact for sums up to 4096; the G pair blocks are
host-padded 24->32 cols for the mandated 16B-aligned pair stride. mod 2 is
an int32 AND on the VectorE. Loads ride the SP HWDGE ring, stores the ACT
ring, so the two rings stream concurrently.
"""

import contextlib

import numpy as np

import concourse.mybir as mybir
from concourse import bacc
from concourse.bass_utils import run_bass_kernel_spmd
from concourse.tile import TileContext

N_CORES = 8
BATCH = 16384
K = 4096
CRC = 24
GW = 32  # G pair block padded to 32 cols: DoubleRow needs 16B-aligned stride
B_SHARD = BATCH // N_CORES  # 2048 rows per core
P = 128
N_CHUNKS = K // P  # 32 k-chunks of 128
DCH = N_CHUNKS // 2  # 16 double-chunks of 256 k (DoubleRow granularity)
RB = 512  # rows per block
N_RB = B_SHARD // RB  # row-blocks per core
MB = 512  # rows per matmul: fp8 moving operand max is 1024 (= 2*MB), and
# each [32, 512] fp32 accumulator exactly fills one PSUM bank
N_MB = RB // MB
FP32 = mybir.dt.float32
FP8 = mybir.dt.float8e4
I32 = mybir.dt.int32
U8 = mybir.dt.uint8

FP8_ONE = 0x38  # float8 e4m3 encoding of 1.0


def _crc_body(
    tc,
    o_d,  # [K + CRC, B_SHARD] uint8, k-major output
    x_d,  # [K, B_SHARD] uint8, k-major input (X.T byte codes)
    g_d,  # [P, DCH * 2 * GW] uint8 packed G
    repeats,
    x_bufs=8,
    pp_bufs=8,
):
    nc = tc.nc
    x_f8 = x_d.bitcast(FP8)
    o_f8 = o_d.bitcast(FP8)
    g_f8 = g_d.bitcast(FP8)

    with contextlib.ExitStack() as stk:
        consts = stk.enter_context(tc.tile_pool(name="consts", bufs=1))
        xpool = stk.enter_context(tc.tile_pool(name="x", bufs=x_bufs))
        pppool = stk.enter_context(tc.tile_pool(name="pp", bufs=pp_bufs, space="PSUM"))
        paripool = stk.enter_context(tc.tile_pool(name="pari", bufs=2))
        parspool = stk.enter_context(tc.tile_pool(name="pars", bufs=2))

        # G host-packed as [128, DCH, 2, 32]: g[p, d, i, m] = G[256d+128i+p, m]
        # (cols 24:32 zero) so each matmul's stationary lhsT is a contiguous
        # DoubleRow slice with a 32B pair stride.
        g_sb = consts.tile([P, DCH, 2, GW], FP8)
        # G rides the store (scalar) ring, which is idle at pass start — on
        # the sync ring it would delay the first X load behind it in FIFO.
        nc.scalar.dma_start(
            out=g_sb,
            in_=g_f8.rearrange("p (d i m) -> p d i m", i=2, m=GW),
        )

        def one_pass():
            # parity for the whole pass, k-major [24, 2048]; stored once
            pars = parspool.tile([CRC, B_SHARD], FP8, tag="pars")
            for b in range(N_RB):
                cols = slice(b * RB, (b + 1) * RB)
                # [128 k, 32 chunks, RB rows] fp8 — X.T block rides through
                # SBUF once: matmul rhs and store source alike.
                x2 = xpool.tile([P, N_CHUNKS, RB], FP8, tag="x2")
                nc.sync.dma_start(
                    out=x2,
                    in_=x_f8[:, cols].rearrange("(c p) r -> p c r", p=P),
                )
                # ppT = sum_d G_d.T @ XT_d = (X @ G).T (rows 24:32 zero);
                # one accumulation group per 512-row sub-slice / PSUM bank
                ppT = pppool.tile([GW, RB], FP32)
                for d in range(DCH):
                    for s in range(N_MB):
                        nc.tensor.matmul(
                            ppT[:, s * MB : (s + 1) * MB],
                            g_sb[:, d],
                            x2[:, 2 * d : 2 * d + 2, s * MB : (s + 1) * MB],
                            start=(d == 0),
                            stop=(d == DCH - 1),
                            perf_mode=mybir.MatmulPerfMode.DoubleRow,
                        )
                # mod-2 of exact-integer f32 sums: cast i32, AND 1, to fp8
                pari = paripool.tile([CRC, RB], I32)
                nc.vector.tensor_copy(pari, ppT[0:CRC])
                nc.vector.tensor_scalar(
                    pari, pari, 1, None, mybir.AluOpType.bitwise_and
                )
                nc.vector.tensor_copy(pars[:, cols], pari)
                nc.scalar.dma_start(
                    out=o_f8[0:K, cols].rearrange("(c p) r -> p c r", p=P),
                    in_=x2,
                )
            nc.scalar.dma_start(out=o_f8[K : K + CRC, :], in_=pars)

        if repeats == 1:
            one_pass()
        else:
            with tc.For_i(0, repeats, 1):
                one_pass()


def pack_g(g_mat: np.ndarray) -> np.ndarray:
    """[4096, 24] {0,1} -> fp8-coded uint8 [128, DCH*2*32] with
    g[p, (d, i), 0:24] = G[256d+128i+p, :] and cols 24:32 zero."""
    g = np.asarray(g_mat)
    gp = np.zeros((P, DCH, 2, GW), np.uint8)
    gp[:, :, :, :CRC] = (
        g.reshape(DCH, 2, P, CRC).transpose(2, 0, 1, 3) != 0
    ).astype(np.uint8) * FP8_ONE
    return np.ascontiguousarray(gp.reshape(P, DCH * 2 * GW))


def encode_xt(x_shard: np.ndarray) -> np.ndarray:
    """{0,1} float32 [rows, K] -> k-major fp8 byte codes [K, rows] uint8."""
    return np.ascontiguousarray(x_shard.astype(np.uint8).T * FP8_ONE)


def decode_out_t(out_u8: np.ndarray) -> np.ndarray:
    """k-major fp8 byte codes [K+CRC, rows] back to {0,1} f32 [rows, K+CRC]."""
    return (out_u8.T != 0).astype(np.float32)


def build_crc_module(repeats: int = 1):
    nc = bacc.Bacc(
        "TRN2", target_bir_lowering=False, debug=False, num_devices=N_CORES
    )
    x_d = nc.dram_tensor("inputs_t", [K, B_SHARD], U8, kind="ExternalInput").ap()
    g_d = nc.dram_tensor(
        "g_packed", [P, DCH * 2 * GW], U8, kind="ExternalInput"
    ).ap()
    o_d = nc.dram_tensor(
        "out_t", [K + CRC, B_SHARD], U8, kind="ExternalOutput"
    ).ap()
    with TileContext(nc) as tc:
        _crc_body(tc, o_d, x_d, g_d, repeats)
    nc.compile()
    return nc


_NC_CACHE = None


def kernel(inputs: np.ndarray, g_mat: np.ndarray) -> np.ndarray:
    global _NC_CACHE
    if _NC_CACHE is None:
        _NC_CACHE = build_crc_module(repeats=1)
    nc = _NC_CACHE

    x = np.asarray(inputs, dtype=np.float32)
    g = np.asarray(g_mat, dtype=np.float32)
    assert x.shape == (BATCH, K) and g.shape == (K, CRC)
    gp = pack_g(g)

    in_maps = [
        {
            "inputs_t": encode_xt(x[i * B_SHARD : (i + 1) * B_SHARD]),
            "g_packed": gp,
        }
        for i in range(N_CORES)
    ]
    res = run_bass_kernel_spmd(nc, in_maps, core_ids=list(range(N_CORES)))
    out = np.concatenate(
        [decode_out_t(r["out_t"]) for r in res.results], axis=0
    )
    return out


# revision 16
# speedup vs baseline: 1.0493x; 1.0493x over previous
"""CRC24A encoder (nn_CRCEncoder) as a Bass/Tile kernel on 8 Trainium2 NeuronCores.

Computation (per the reference):
    out = concat([X, (X @ G) mod 2], axis=-1)
with X [16384, 4096] of {0,1} float32 and G [4096, 24] of {0,1} float32.

Strategy: pure data parallel over the batch dim — each of the 8 cores gets a
[2048, 4096] shard and the full (replicated) G. Two layout decisions make the
device side a pure streaming kernel at the HBM roofline:

  - 8-bit I/O. Every value is exactly 0.0 or 1.0, which fp8 e4m3 represents
    exactly (0x00 / 0x38), so the device moves 16.8 MiB per core per pass
    instead of 67.3 MiB at f32. The host converts f32 <-> byte codes; DRAM
    tensors are declared uint8 so jax/PJRT never sees an fp8 dtype, and
    device-side APs bitcast to float8e4 where the engines need it.
  - k-major (transposed) layout. The host uploads X.T [4096, 2048] and reads
    back the output k-major [4120, 2048]. Loads then put the contraction dim
    on SBUF partitions directly: no TensorE transposes, no PSUM evacuation
    copies — the parity matmul streams straight from the DMA staging tile,
    and the X passthrough stores bit-verbatim from the same tile. The parity
    (X @ G).T [24, rows] is itself k-major, landing as output rows 4096:4119
    with no transpose-back. Host-side de-transposition happens once per
    kernel() call.

Per 512-row block: 16 DoubleRow fp8 matmuls (contraction 256/instruction,
0.5 cycles/row, moving free dim 512 — the full-win regime) accumulate
(X @ G).T in fp32 PSUM, exact for sums up to 4096; the G pair blocks are
host-padded 24->32 cols for the mandated 16B-aligned pair stride. mod 2 is
an int32 AND on the VectorE. Loads ride the SP HWDGE ring, stores the ACT
ring, so the two rings stream concurrently.
"""

import contextlib

import numpy as np

import concourse.mybir as mybir
from concourse import bacc
from concourse.bass_utils import run_bass_kernel_spmd
from concourse.tile import TileContext

N_CORES = 8
BATCH = 16384
K = 4096
CRC = 24
GW = 32  # G pair block padded to 32 cols: DoubleRow needs 16B-aligned stride
B_SHARD = BATCH // N_CORES  # 2048 rows per core
P = 128
N_CHUNKS = K // P  # 32 k-chunks of 128
DCH = N_CHUNKS // 2  # 16 double-chunks of 256 k (DoubleRow granularity)
RB = 1024  # rows per block: 1 KiB DMA descriptors, well over line-rate minimum
N_RB = B_SHARD // RB  # row-blocks per core
MB = 512  # rows per matmul: fp8 moving operand max is 1024 (= 2*MB), and
# each [32, 512] fp32 accumulator exactly fills one PSUM bank
N_MB = RB // MB
FP32 = mybir.dt.float32
FP8 = mybir.dt.float8e4
I32 = mybir.dt.int32
U8 = mybir.dt.uint8

FP8_ONE = 0x38  # float8 e4m3 encoding of 1.0


def _crc_body(
    tc,
    o_d,  # [K + CRC, B_SHARD] uint8, k-major output
    x_d,  # [K, B_SHARD] uint8, k-major input (X.T byte codes)
    g_d,  # [P, DCH * 2 * GW] uint8 packed G
    repeats,
    x_bufs=4,
    pp_bufs=4,
):
    nc = tc.nc
    x_f8 = x_d.bitcast(FP8)
    o_f8 = o_d.bitcast(FP8)
    g_f8 = g_d.bitcast(FP8)

    with contextlib.ExitStack() as stk:
        consts = stk.enter_context(tc.tile_pool(name="consts", bufs=1))
        xpool = stk.enter_context(tc.tile_pool(name="x", bufs=x_bufs))
        pppool = stk.enter_context(tc.tile_pool(name="pp", bufs=pp_bufs, space="PSUM"))
        paripool = stk.enter_context(tc.tile_pool(name="pari", bufs=2))
        parspool = stk.enter_context(tc.tile_pool(name="pars", bufs=2))

        # G host-packed as [128, DCH, 2, 32]: g[p, d, i, m] = G[256d+128i+p, m]
        # (cols 24:32 zero) so each matmul's stationary lhsT is a contiguous
        # DoubleRow slice with a 32B pair stride.
        g_sb = consts.tile([P, DCH, 2, GW], FP8)
        # G rides the store (scalar) ring, which is idle at pass start — on
        # the sync ring it would delay the first X load behind it in FIFO.
        nc.scalar.dma_start(
            out=g_sb,
            in_=g_f8.rearrange("p (d i m) -> p d i m", i=2, m=GW),
        )

        def one_pass():
            # parity for the whole pass, k-major [24, 2048]; stored once
            pars = parspool.tile([CRC, B_SHARD], FP8, tag="pars")
            for b in range(N_RB):
                cols = slice(b * RB, (b + 1) * RB)
                # [128 k, 32 chunks, RB rows] fp8 — X.T block rides through
                # SBUF once: matmul rhs and store source alike.
                x2 = xpool.tile([P, N_CHUNKS, RB], FP8, tag="x2")
                nc.sync.dma_start(
                    out=x2,
                    in_=x_f8[:, cols].rearrange("(c p) r -> p c r", p=P),
                )
                # ppT = sum_d G_d.T @ XT_d = (X @ G).T (rows 24:32 zero);
                # one accumulation group per 512-row sub-slice / PSUM bank
                ppT = pppool.tile([GW, RB], FP32)
                for d in range(DCH):
                    for s in range(N_MB):
                        nc.tensor.matmul(
                            ppT[:, s * MB : (s + 1) * MB],
                            g_sb[:, d],
                            x2[:, 2 * d : 2 * d + 2, s * MB : (s + 1) * MB],
                            start=(d == 0),
                            stop=(d == DCH - 1),
                            perf_mode=mybir.MatmulPerfMode.DoubleRow,
                        )
                # mod-2 of exact-integer f32 sums: cast i32, AND 1, to fp8
                pari = paripool.tile([CRC, RB], I32)
                nc.vector.tensor_copy(pari, ppT[0:CRC])
                nc.vector.tensor_scalar(
                    pari, pari, 1, None, mybir.AluOpType.bitwise_and
                )
                nc.vector.tensor_copy(pars[:, cols], pari)
                nc.scalar.dma_start(
                    out=o_f8[0:K, cols].rearrange("(c p) r -> p c r", p=P),
                    in_=x2,
                )
            nc.scalar.dma_start(out=o_f8[K : K + CRC, :], in_=pars)

        if repeats == 1:
            one_pass()
        else:
            with tc.For_i(0, repeats, 1):
                one_pass()


def pack_g(g_mat: np.ndarray) -> np.ndarray:
    """[4096, 24] {0,1} -> fp8-coded uint8 [128, DCH*2*32] with
    g[p, (d, i), 0:24] = G[256d+128i+p, :] and cols 24:32 zero."""
    g = np.asarray(g_mat)
    gp = np.zeros((P, DCH, 2, GW), np.uint8)
    gp[:, :, :, :CRC] = (
        g.reshape(DCH, 2, P, CRC).transpose(2, 0, 1, 3) != 0
    ).astype(np.uint8) * FP8_ONE
    return np.ascontiguousarray(gp.reshape(P, DCH * 2 * GW))


def encode_xt(x_shard: np.ndarray) -> np.ndarray:
    """{0,1} float32 [rows, K] -> k-major fp8 byte codes [K, rows] uint8."""
    return np.ascontiguousarray(x_shard.astype(np.uint8).T * FP8_ONE)


def decode_out_t(out_u8: np.ndarray) -> np.ndarray:
    """k-major fp8 byte codes [K+CRC, rows] back to {0,1} f32 [rows, K+CRC]."""
    return (out_u8.T != 0).astype(np.float32)


def build_crc_module(repeats: int = 1):
    nc = bacc.Bacc(
        "TRN2", target_bir_lowering=False, debug=False, num_devices=N_CORES
    )
    x_d = nc.dram_tensor("inputs_t", [K, B_SHARD], U8, kind="ExternalInput").ap()
    g_d = nc.dram_tensor(
        "g_packed", [P, DCH * 2 * GW], U8, kind="ExternalInput"
    ).ap()
    o_d = nc.dram_tensor(
        "out_t", [K + CRC, B_SHARD], U8, kind="ExternalOutput"
    ).ap()
    with TileContext(nc) as tc:
        _crc_body(tc, o_d, x_d, g_d, repeats)
    nc.compile()
    return nc


_NC_CACHE = None


def kernel(inputs: np.ndarray, g_mat: np.ndarray) -> np.ndarray:
    global _NC_CACHE
    if _NC_CACHE is None:
        _NC_CACHE = build_crc_module(repeats=1)
    nc = _NC_CACHE

    x = np.asarray(inputs, dtype=np.float32)
    g = np.asarray(g_mat, dtype=np.float32)
    assert x.shape == (BATCH, K) and g.shape == (K, CRC)
    gp = pack_g(g)

    in_maps = [
        {
            "inputs_t": encode_xt(x[i * B_SHARD : (i + 1) * B_SHARD]),
            "g_packed": gp,
        }
        for i in range(N_CORES)
    ]
    res = run_bass_kernel_spmd(nc, in_maps, core_ids=list(range(N_CORES)))
    out = np.concatenate(
        [decode_out_t(r["out_t"]) for r in res.results], axis=0
    )
    return out
